# revision 44
# baseline (speedup 1.0000x reference)
"""Trainium2 Bass kernel for the DFBL (Gabor filterbank + Kaiser pooling + PCEN) model.

Contract: kernel(**inputs) takes the FULL unsharded inputs
(x [8,1,160000], six [64] param vectors) and returns the FULL output
[8, 64, 1000] float32. Internally shards batch across 8 NeuronCores.

Algorithm (per core, one batch element):
  1. Gabor conv as fp8 DoubleRow matmuls via the residue decomposition
     t = 128u + s: out[n, 128u+s] = sum_d Wsd[q,n].T @ x2[q, u+d], with
     an x-residual correction for fp8 accuracy (x = x8 + r16/16;
     y = W8@x8 + (W8/16)@r16; rel err ~7e-3 end to end). x2 holds 4 fp8
     planes [x8, x8<<1, r16, r16<<1] so DoubleRow (d, d+1) k-tile pairs
     read adjacent planes without overlapping APs.
  2. |.|^2 on ACT (the only engine that can square a PSUM operand),
     scaled by 1/64 into fp8 range, stored s-minor in a double-buffered
     fp8 segment buffer so segment k+1's conv overlaps segment k's
     pooling phase.
  3. Kaiser pooling on the PE: fold-matmul [chan,time] -> [time, 64]
     chunks (the rhs is a [128, 64] fold matrix F[p, c] = (p % 64 == c),
     so real^2 + imag^2 are summed while transposing at half the
     columns), then fp8 DoubleRow banded-kaiser matmuls accumulate
     pooledT[tp, 64] in two PSUM tiles (even/odd block interleave:
     temporally-overlapping blocks must not share a tile, or a block's
     start=True pends the whole 2KB zero-region and restarts its
     neighbour's open accumulation).
  4. PCEN scan as a decay-matrix matmul ema = pooled @ L (L prefetched
     during segment 1), then a 2-half-pipelined elementwise pow chain
     on ACT/DVE.
"""

import math
import os

import ml_dtypes
import numpy as np

SR = 16000
NF = 64
GK = 401
PK = 401
PSTRIDE = 160
PCEN_S = 0.025
FMIN = 30.0
FMAX = SR / 2.0 * 0.5
B, T = 8, 160000
TP = 1000
U = 1250  # T / 128
X2C = 1254  # x2 columns: u+d+2 for u<1250, d in [-2,2]
SEG_BOUNDS = [(0, 313), (313, 626), (626, 939), (939, 1250)]
N_CORES = 8

BF16 = ml_dtypes.bfloat16

# exposed for test.py
LAST_RESULT = None
LAST_NC = None
LAST_IN_MAPS = None


# ----------------------------------------------------------------- host math

def _softplus(x):
    return np.logaddexp(0.0, x)


def _host_filters(p_center, p_bw):
    """Wcat [128, 401] f32: rows 0-63 real, 64-127 imag, scaled by sqrt(0.5)."""
    half = (GK - 1) // 2
    t = np.arange(-half, half + 1, dtype=np.float64) / SR
    fc = np.clip(np.exp(p_center.astype(np.float64)), FMIN, FMAX - 10.0)
    bw_pos = _softplus(p_bw.astype(np.float64)) * 1000.0
    max_bw = 2.0 * np.minimum(fc - FMIN, FMAX - fc)
    bw = np.minimum(bw_pos, np.maximum(max_bw, 50.0))
    f_low = np.maximum(fc - 0.5 * bw, FMIN)
    f_high = np.minimum(fc + 0.5 * bw, FMAX)
    sigma = 0.5 / np.maximum(f_high - f_low, 20.0)
    env = np.exp(-0.5 * (t[None, :] / sigma[:, None]) ** 2)
    phase = 2.0 * np.pi * fc[:, None] * t[None, :]
    real_k = env * np.cos(phase)
    imag_k = env * np.sin(phase)
    W = np.concatenate([real_k, imag_k], axis=0) * np.sqrt(0.5)
    return W.astype(np.float32)


def _host_kaiser(beta):
    b = np.clip(beta.astype(np.float64), 1.0, 20.0)
    n = np.arange(PK, dtype=np.float64)
    arg = b[:, None] * np.sqrt(1.0 - (2.0 * n[None, :] / (PK - 1.0) - 1.0) ** 2)
    kais = np.i0(arg) / (np.i0(b)[:, None] + 1e-8)
    return kais.astype(np.float32)


def _valid_d(s):
    lo = int(math.ceil((s - 327) / 128))
    hi = (s + 200) // 128
    return list(range(lo, hi + 1))


def _build_weight_array(W, dt=None):
    """W_all [128, ntiles*128], tiles ordered (s asc, d asc); returns
    (W_all, offsets) with offsets[s] = first tile index of s."""
    tiles = []
    offsets = []
    for s in range(128):
        offsets.append(len(tiles))
        for d in _valid_d(s):
            tile = np.zeros((128, 128), np.float32)
            q = np.arange(128)
            k = 128 * d + q + 200 - s
            msk = (k >= 0) & (k < GK)
            tile[msk, :] = W[:, k[msk]].T
            tiles.append(tile)
    W_all = np.concatenate(tiles, axis=1).astype(dt or BF16)
    return W_all, offsets


def _build_kt_array(kr):
    """KT [128, 163*128] fp8e4m3; tile index o+2 for offset o in [-2, 160]:
    KT_o[q, m] = kr[128*o + q - 160*m + 200] (0 outside [0, 401))."""
    tiles = []
    for o in range(-2, 161):
        tile = np.zeros((128, 128), np.float32)
        for m in range(128):
            base = 128 * o - 160 * m + 200
            ks = np.arange(128) + base
            msk = (ks >= 0) & (ks < PK)
            tile[msk, m] = kr[ks[msk]]
        tiles.append(tile)
    return np.concatenate(tiles, axis=1).astype(ml_dtypes.float8_e4m3)


def _build_L():
    k_idx = np.arange(1024)
    tp_idx = np.arange(TP)
    Lm = np.where(
        (k_idx[:, None] <= tp_idx[None, :]) & (k_idx[:, None] < TP),
        PCEN_S * (1.0 - PCEN_S) ** np.clip(tp_idx[None, :] - k_idx[:, None], 0, None),
        0.0,
    )
    return Lm.astype(np.float32)


def _pool_blocks(c):
    """pooled blocks touched by time-chunk c."""
    b_lo = max(0, int(math.ceil((c - 160) / 160)))
    b_hi = min(7, (c + 2) // 160)
    return list(range(b_lo, b_hi + 1))


# ------------------------------------------------------------- device kernel

def _build_program():
    import concourse.bacc as bacc
    import concourse.bass as bass
    import concourse.mybir as mybir
    import concourse.tile as tile
    from concourse._compat import axon_active

    f32 = mybir.dt.float32
    bf16 = mybir.dt.bfloat16
    AF = mybir.ActivationFunctionType
    ALU = mybir.AluOpType

    n_wtiles = sum(len(_valid_d(s)) for s in range(128))
    woff = []
    acc = 0
    for s in range(128):
        woff.append(acc)
        acc += len(_valid_d(s))

    nc = bacc.Bacc(
        "TRN2",
        target_bir_lowering=False,
        debug=not axon_active(),
        num_devices=N_CORES,
    )

    # packed constants: CB = [W8 fp8 | W8r fp8 | KT fp8] byte-packed into a
    # bf16 blob, plus FLD bf16; CF = [L(8 blk-major) | IDF | PAR] f32 — fewer
    # PJRT args per execute (the axon per-exec dispatch cost scales with
    # buffer count). The conv runs in fp8 DoubleRow with an x-residual
    # correction: x = x8 + r16/16, y = W8@x8 + (W8/16)@r16.
    fp8 = mybir.dt.float8e4
    W8R_OFF = n_wtiles * 64  # bf16 cols; fp8 cols are 2x
    KT_OFF = n_wtiles * 128
    FLD_OFF = KT_OFF + (163 * 128) // 2
    CB_COLS = FLD_OFF + 64
    IDF_OFF = 8 * TP
    PAR_OFF = IDF_OFF + 128
    CF_COLS = PAR_OFF + 5

    # x2 carries 4 fp8 planes: [x8, x8<<1col, r16, r16<<1col]
    x2_d = nc.dram_tensor("x2", [128, 4 * X2C], fp8, kind="ExternalInput").ap()
    cb_d = nc.dram_tensor("CB", [128, CB_COLS], bf16, kind="ExternalInput").ap()
    cf_d = nc.dram_tensor("CF", [128, CF_COLS], f32, kind="ExternalInput").ap()
    y_d = nc.dram_tensor("Y", [64, TP], f32, kind="ExternalOutput").ap()
    wa_d = cb_d[:, 0:W8R_OFF].bitcast(fp8)
    wb_d = cb_d[:, W8R_OFF:KT_OFF].bitcast(fp8)
    kt_d = cb_d[:, KT_OFF:FLD_OFF].bitcast(fp8)
    fld_d = cb_d[:, FLD_OFF : FLD_OFF + 64]
    idf_d = cf_d[:, IDF_OFF : IDF_OFF + 128]
    par_d = cf_d[0:64, PAR_OFF : PAR_OFF + 5]

    # first/last pooling contribution per pooled block, for start/stop flags
    blk_first = {}
    blk_last = {}
    for c in range(U):
        for blk in _pool_blocks(c):
            if blk not in blk_first:
                blk_first[blk] = c
            blk_last[blk] = c

    with tile.TileContext(nc) as tc:
        with (
            tc.tile_pool(name="const", bufs=1) as const_pool,
            tc.tile_pool(name="w", bufs=3) as wpool,
            tc.tile_pool(name="sq", bufs=2) as sq_pool,
            tc.tile_pool(name="sct", bufs=6) as sct_pool,

            tc.tile_pool(name="misc", bufs=1) as misc_pool,
            tc.tile_pool(name="psA", bufs=4, space="PSUM") as psA,
            tc.tile_pool(name="psB", bufs=2, space="PSUM") as psB,
            tc.tile_pool(name="psC", bufs=1, space="PSUM") as psC,
        ):
            x2_sb = const_pool.tile([128, 4, X2C], fp8, tag="x2")
            nc.sync.dma_start(x2_sb[:], x2_d[:])
            kt_sb = const_pool.tile([128, 163 * 128], fp8, tag="kt")
            fld_sb = const_pool.tile([128, 64], bf16, tag="fld")
            idf_sb = const_pool.tile([128, 128], f32, tag="idf")
            par_sb = const_pool.tile([64, 5], f32, tag="par")
            l_sb = const_pool.tile([128, 8 * TP], f32, tag="lsb")

            # two PSUM tiles, even/odd block interleave: temporally-overlapping
            # blocks (b-1, b) must not share a PSUM tile, or b's start=True
            # (which pends the whole 2KB zero-region) restarts b-1's
            # still-open accumulation
            pooled_ps = [
                psC.tile([128, 256], f32, tag=f"pool{i}", name=f"pool{i}")
                for i in range(2)
            ]

            # squares: ACT majority (Square reads PSUM once); DVE offload uses
            # a two-op form (scaled PSUM->SBUF copy, then SBUF x SBUF mul)
            # since no engine may read a PSUM operand twice and GPSIMD has no
            # PSUM port at all.
            sq_route = [0] * 128

            for (u0, u1) in SEG_BOUNDS:
                useg = u1 - u0
                # fp8 squares: halves the SBUF footprint so two segment
                # buffers fit, letting segment k+1's conv overlap segment k's
                # (DVE-bound) pooling phase
                sq_seg = sq_pool.tile([128, 313 * 128], fp8, tag="sq", name="sq")
                sq_view = sq_seg[:].rearrange("p (u s) -> p u s", s=128)

                GS = 8
                for g in range(0, 128, GS):
                    g_lo = woff[g]
                    g_hi = woff[g + GS] if g + GS < 128 else n_wtiles
                    gw = g_hi - g_lo
                    wt = wpool.tile([128, 2, 40 * 128], fp8, tag="w", name="wt")
                    nc.sync.dma_start(
                        wt[:, 0, 0 : gw * 128],
                        wa_d[:, g_lo * 128 : g_hi * 128],
                    )
                    nc.sync.dma_start(
                        wt[:, 1, 0 : gw * 128],
                        wb_d[:, g_lo * 128 : g_hi * 128],
                    )
                    for s in range(g, g + GS):
                        ds = _valid_d(s)
                        nt = len(ds)
                        toff = woff[s] - g_lo
                        cps = psA.tile([128, useg], f32, tag="conv", name="cps")
                        # two fp8 streams (x8 planes 0/1, r16 planes 2/3),
                        # DoubleRow-paired over adjacent d k-tiles
                        insts = []
                        for strm in range(2):
                            i = 0
                            while i < nt:
                                if i + 1 < nt:
                                    insts.append((strm, i, True))
                                    i += 2
                                else:
                                    insts.append((strm, i, False))
                                    i += 1
                        for k, (strm, i, dr) in enumerate(insts):
                            d = ds[i]
                            col = u0 + d + 2
                            if dr:
                                nc.tensor.matmul(
                                    cps[:],
                                    lhsT=wt[
                                        :, strm, (toff + i) * 128 : (toff + i + 2) * 128
                                    ].rearrange("p (k m) -> p k m", k=2),
                                    rhs=x2_sb[
                                        :, 2 * strm : 2 * strm + 2, col : col + useg
                                    ],
                                    start=(k == 0),
                                    stop=(k == len(insts) - 1),
                                    perf_mode=mybir.MatmulPerfMode.DoubleRow,
                                )
                            else:
                                nc.tensor.matmul(
                                    cps[:],
                                    lhsT=wt[
                                        :, strm, (toff + i) * 128 : (toff + i + 1) * 128
                                    ],
                                    rhs=x2_sb[:, 2 * strm, col : col + useg],
                                    start=(k == 0),
                                    stop=(k == len(insts) - 1),
                                )
                        # scale=0.125 -> sq holds y^2/64, keeping the folded
                        # sums within fp8e4's range for the pooling stage (the
                        # PCEN ratio is scale-free; only eps is compensated)
                        dst = sq_view[:, 0:useg, s : s + 1]
                        if sq_route[s] == 0:
                            nc.scalar.activation(dst, cps[:], AF.Square, scale=0.125)
                        else:
                            ysc = sct_pool.tile(
                                [128, 313], bf16, tag="ysc", name="ysc"
                            )
                            nc.vector.tensor_scalar(
                                ysc[:, 0:useg], cps[:], 0.125, None, ALU.mult
                            )
                            nc.vector.tensor_mul(
                                dst, ysc[:, 0:useg], ysc[:, 0:useg]
                            )

                if u0 == 0:
                    # deferred const loads: queued after segment-0 conv weights
                    # so the first weight group isn't stuck behind 5.5 MB
                    nc.sync.dma_start(kt_sb[:], kt_d[:])
                    nc.sync.dma_start(fld_sb[:], fld_d[:])
                    nc.sync.dma_start(idf_sb[:], idf_d[:])
                    nc.sync.dma_start(par_sb[:], par_d[:])
                elif u0 == SEG_BOUNDS[1][0]:
                    # prefetch L during segment 1 so the PCEN tail has no DMA
                    nc.sync.dma_start(l_sb[:], cf_d[:, 0 : 8 * TP])
                for cbase in range(u0, u1, 4):
                    n4 = min(4, u1 - cbase)
                    tp_ps = psB.tile([128, 256], f32, tag="tp", name="tpps")
                    for j in range(n4):
                        cc = cbase - u0 + j
                        # fold-transpose: sq_chunk.T @ F sums real^2+imag^2
                        # while transposing (F is not a permutation, so this
                        # must be a regular matmul, not transpose mode)
                        nc.tensor.matmul(
                            tp_ps[:, j * 64 : (j + 1) * 64],
                            lhsT=sq_seg[:, cc * 128 : (cc + 1) * 128],
                            rhs=fld_sb[:],
                            start=True,
                            stop=True,
                        )
                    sct = sct_pool.tile([128, 256], fp8, tag="sct", name="sct")
                    nc.vector.tensor_copy(
                        sct[:, 0 : n4 * 64], tp_ps[:, 0 : n4 * 64]
                    )
                    # fp8 DoubleRow pooling: pair adjacent chunks hitting the
                    # same block (their kaiser tiles are adjacent in kt_sb)
                    blk_js = {}
                    for j in range(n4):
                        for blk in _pool_blocks(cbase + j):
                            blk_js.setdefault(blk, []).append(j)
                    for blk, js in blk_js.items():
                        col = (blk // 2) * 64
                        out_ap = pooled_ps[blk % 2][:, col : col + 64]
                        i = 0
                        while i < len(js):
                            j0 = js[i]
                            c0 = cbase + j0
                            o = c0 - 160 * blk
                            if i + 1 < len(js) and js[i + 1] == j0 + 1:
                                nc.tensor.matmul(
                                    out_ap,
                                    lhsT=kt_sb[
                                        :, (o + 2) * 128 : (o + 4) * 128
                                    ].rearrange("p (k m) -> p k m", k=2),
                                    rhs=sct[
                                        :, j0 * 64 : (j0 + 2) * 64
                                    ].rearrange("p (k m) -> p k m", k=2),
                                    start=(blk_first[blk] == c0),
                                    stop=(blk_last[blk] == c0 + 1),
                                    perf_mode=mybir.MatmulPerfMode.DoubleRow,
                                    skip_group_check=True,
                                )
                                i += 2
                            else:
                                nc.tensor.matmul(
                                    out_ap,
                                    lhsT=kt_sb[:, (o + 2) * 128 : (o + 3) * 128],
                                    rhs=sct[:, j0 * 64 : (j0 + 1) * 64],
                                    start=(blk_first[blk] == c0),
                                    stop=(blk_last[blk] == c0),
                                    skip_group_check=True,
                                )
                                i += 1

            # ---- PCEN tail ----
            poolsumT = misc_pool.tile([128, 512], f32, tag="pst")
            for blk in range(8):
                col = (blk // 2) * 64
                nc.vector.tensor_copy(
                    poolsumT[:, blk * 64 : (blk + 1) * 64],
                    pooled_ps[blk % 2][:, col : col + 64],
                )

            ema_ps = [psA.tile([64, 500], f32, tag="conv", name=f"ema{_i}") for _i in range(2)]
            for blk in range(8):
                for half in range(2):
                    nc.tensor.matmul(
                        ema_ps[half][:],
                        lhsT=poolsumT[:, blk * 64 : (blk + 1) * 64],
                        rhs=l_sb[:, blk * TP + half * 500 : blk * TP + (half + 1) * 500],
                        start=(blk == 0),
                        stop=(blk == 7),
                    )

            pnm_ps = [psB.tile([64, 512], f32, tag="tp", name=f"pnm{_i}") for _i in range(2)]
            for blk in range(8):
                nc.tensor.transpose(
                    pnm_ps[blk // 4][:, (blk % 4) * 128 : (blk % 4 + 1) * 128],
                    poolsumT[:, blk * 64 : (blk + 1) * 64],
                    idf_sb[:],
                )

            t0 = misc_pool.tile([64, TP], f32, tag="t0")
            rec = misc_pool.tile([64, TP], f32, tag="rec")
            pnm = misc_pool.tile([64, TP], f32, tag="pnm")
            t2 = misc_pool.tile([64, TP], f32, tag="t2")
            t3 = misc_pool.tile([64, TP], f32, tag="t3")
            t4 = misc_pool.tile([64, TP], f32, tag="t4")
            y_sb = misc_pool.tile([64, TP], f32, tag="y")
            nc.scalar.copy(pnm[:, 0:512], pnm_ps[0][:])
            nc.scalar.copy(pnm[:, 512:TP], pnm_ps[1][:, 0:488])
            # 2-half pipeline: each half's ACT/DVE chain overlaps the other's
            for half in range(2):
                sl = slice(half * 500, (half + 1) * 500)
                nc.scalar.activation(
                    t0[:, sl], ema_ps[half][:], AF.Identity, bias=par_sb[:, 4:5]
                )
                nc.vector.reciprocal(rec[:, sl], t0[:, sl])
                nc.vector.tensor_mul(t2[:, sl], pnm[:, sl], rec[:, sl])
                nc.scalar.activation(
                    t3[:, sl], t2[:, sl], AF.Ln, bias=par_sb[:, 0:1], scale=1.0
                )
                nc.scalar.activation(
                    t4[:, sl], t3[:, sl], AF.Exp, bias=0.0, scale=par_sb[:, 1:2]
                )
                nc.vector.tensor_scalar(
                    y_sb[:, sl], t4[:, sl], par_sb[:, 2:3], par_sb[:, 3:4],
                    ALU.mult, ALU.subtract,
                )
                nc.sync.dma_start(y_d[:, sl], y_sb[:, sl])

    nc.compile()
    return nc


def _numpy_fallback(x, W, kais, pcen_g, pcen_o, pcen_e):
    """Correct-but-slow host path for non-uniform beta (never hit with the
    harness inputs, which use a uniform beta)."""
    out = np.zeros((B, NF, TP), np.float32)
    Wr, Wi = W[:NF] / np.sqrt(0.5), W[NF:] / np.sqrt(0.5)
    for b in range(B):
        xp = np.zeros(T + 2 * 200, np.float32)
        xp[200 : 200 + T] = x[b, 0]
        win = np.lib.stride_tricks.sliding_window_view(xp, GK)  # [T, GK]
        real = win @ Wr.T
        imag = win @ Wi.T
        scal = 0.5 * (real ** 2 + imag ** 2)  # [T, NF]
        sp = np.zeros((T + 2 * 200, NF), np.float32)
        sp[200 : 200 + T] = scal
        pooled = np.zeros((TP, NF), np.float32)
        for tp in range(TP):
            seg = sp[tp * PSTRIDE : tp * PSTRIDE + PK]
            pooled[tp] = np.einsum("kn,nk->n", seg, kais)
        g = np.clip(pcen_g, 0.5, 0.999)
        o = np.clip(pcen_o, 0.0, 10.0)
        e = np.clip(pcen_e, 0.1, 1.0)
        ema = np.zeros(NF, np.float32)
        for tp in range(TP):
            ema = (1.0 - PCEN_S) * ema + PCEN_S * pooled[tp]
            out[b, :, tp] = ((pooled[tp] / (ema + 1e-6) + o) ** e - o ** e) * g
    return out


def kernel(x, p_center, p_bw, beta, pcen_g, pcen_o, pcen_e):
    global LAST_RESULT
    x = np.asarray(x, np.float32)
    p_center = np.asarray(p_center, np.float32)
    p_bw = np.asarray(p_bw, np.float32)
    beta = np.asarray(beta, np.float32)
    pcen_g = np.asarray(pcen_g, np.float32)
    pcen_o = np.asarray(pcen_o, np.float32)
    pcen_e = np.asarray(pcen_e, np.float32)

    W = _host_filters(p_center, p_bw)
    kais = _host_kaiser(beta)
    if not np.all(kais == kais[0:1]):
        return _numpy_fallback(x, W, kais, pcen_g, pcen_o, pcen_e)

    E4M3 = ml_dtypes.float8_e4m3
    W8_all, _ = _build_weight_array(W, dt=E4M3)
    W8r_all, _ = _build_weight_array(W / 16.0, dt=E4M3)
    KT = _build_kt_array(kais[0])
    Lm = _build_L()
    g = np.clip(pcen_g, 0.5, 0.999)
    o = np.clip(pcen_o, 0.0, 10.0)
    e = np.clip(pcen_e, 0.1, 1.0)
    # eps scaled by 1/64: the squares are stored as y^2/64 (fp8 range), which
    # scales pooled and ema identically; only the eps offset needs matching
    par = np.stack(
        [o, e, g, g * o ** e, np.full(NF, 1e-6 / 64.0, np.float32)], axis=1
    ).astype(np.float32)
    fld = (np.arange(128)[:, None] % 64 == np.arange(64)[None, :]).astype(
        np.float32
    ).astype(BF16)
    idf = np.eye(128, dtype=np.float32)

    # packed constant blobs (must mirror _build_program's offsets);
    # W8/W8r/KT are fp8 byte-packed into the bf16 blob
    W8_bf = np.ascontiguousarray(W8_all).view(BF16)
    W8r_bf = np.ascontiguousarray(W8r_all).view(BF16)
    KT_bf = np.ascontiguousarray(KT).view(BF16)
    CB = np.concatenate([W8_bf, W8r_bf, KT_bf, fld], axis=1)
    L_bm = np.concatenate(
        [Lm[blk * 128 : (blk + 1) * 128, :] for blk in range(8)], axis=1
    )  # [128, 8000] blk-major
    par_pad = np.zeros((128, 5), np.float32)
    par_pad[:64] = par
    CF = np.concatenate([L_bm, idf, par_pad], axis=1).astype(np.float32)

    def _plane(v):
        xp = np.zeros(128 * X2C, np.float32)
        xp[256 : 256 + T] = v
        return np.ascontiguousarray(xp.reshape(X2C, 128).T)

    x2s = []
    for b in range(B):
        x8 = x[b, 0].astype(E4M3).astype(np.float32)
        r16 = ((x[b, 0] - x8) * 16.0).astype(E4M3).astype(np.float32)
        pl = _plane(x8)
        plr = _plane(r16)
        pls = np.zeros_like(pl)
        pls[:, :-1] = pl[:, 1:]
        plrs = np.zeros_like(plr)
        plrs[:, :-1] = plr[:, 1:]
        # planes: [x8, x8 shifted 1 col, r16, r16 shifted 1 col] so DoubleRow
        # (d, d+1) pairs read adjacent planes without overlapping APs
        x2q = np.stack([pl, pls, plr, plrs], axis=1).astype(E4M3)
        x2s.append(x2q.reshape(128, 4 * X2C))

    nc = _build_program()

    shared = {"CB": CB, "CF": CF}
    in_maps = [dict(shared, x2=x2s[b]) for b in range(B)]
    global LAST_NC, LAST_IN_MAPS
    LAST_NC = nc
    LAST_IN_MAPS = in_maps

    from concourse.bass_utils import run_bass_kernel_spmd

    trace = bool(int(os.environ.get("DFBL_TRACE", "0")))
    res = run_bass_kernel_spmd(
        nc, in_maps, list(range(N_CORES)), trace=trace
    )
    LAST_RESULT = res
    out = np.stack([res.results[b]["Y"] for b in range(B)], axis=0)
    return out.astype(np.float32)

